# revision 16
# baseline (speedup 1.0000x reference)
"""Multi-head attention (B=2, S=2048, D=1024, H=16) on 8 trn2 NeuronCores.

Sharding: batch (2) x head-groups (4 heads each, 4 groups) = 8 cores.
Each core computes Q/K/V projections for its 4 heads on its batch,
causal-masked softmax attention, and a partial output projection
(row-sharded w_o); the host sums the 4 partials per batch.

Layout strategy: the host stages every input pre-permuted so each DMA is
128 contiguous per-partition descriptors (xP[p, kt, s] = x.T[kt*128+p, s]),
and x is transferred in 8 contraction chunks so the chunk-major projection
matmuls start as soon as chunk 0 lands.  Attention scores are computed
transposed (ST[k, q]) so P = exp(ST) feeds the PV matmul directly, and V's
64 pad columns are ALL ones, which makes the PV matmul replicate the
softmax denominator across partitions 64:128 -- normalization is then just
reciprocal_approx_fast + tensor_mul, no cross-partition broadcast.
PSUM->SBUF copies are split between the Scalar and Vector engines so the
Activation engine's only phase-B job is exp (the pipeline pacer); y is
written back in bf16 per 128-row tile.
"""
import sys

sys.path.insert(0, "/opt/trn_rl_repo")

import numpy as np
import ml_dtypes

import concourse.bass as bass
import concourse.mybir as mybir
import concourse.tile as tile
from concourse.bass_utils import run_bass_kernel_spmd

B, S, D, H, DK = 2, 2048, 1024, 16, 64
NCORES = 8
HG = 4                # heads per core
DHG = HG * DK         # 256 head-dims per core
KT = D // 128         # 8 contraction chunks for the projections
ST128 = S // 128      # 16 128-row tiles of S
QS = 512              # q-strip width
NQS = S // QS         # 4 strips

f32 = mybir.dt.float32
bf16 = mybir.dt.bfloat16
EXP = mybir.ActivationFunctionType.Exp
LN = mybir.ActivationFunctionType.Ln


def _split_waits(nc, max_waits=1):
    """This walrus build rejects >1 SyncWait per instruction (and >0 on
    fp32-family matmuls, which lower through the 1-wait S3_LW struct).
    Hoist excess waits onto dedicated NOPs on the same engine queue."""
    n = 0
    for fn in nc.m.functions:
        for blk in fn.blocks:
            new = []
            for ins in blk.instructions:
                si = getattr(ins, "sync_info", None)
                if si is not None and si.on_wait:
                    limit = 0 if isinstance(ins, mybir.InstMatmult) else max_waits
                    if len(si.on_wait) > limit:
                        waits = list(si.on_wait)
                        hoist = waits if limit == 0 else waits[:-limit]
                        keep = [] if limit == 0 else waits[-limit:]
                        for w in hoist:
                            n += 1
                            new.append(
                                mybir.InstNoOp(
                                    name=f"I-waitfix-{n}",
                                    engine=ins.engine,
                                    bass_nofuse=True,
                                    sync_info=mybir.SyncInfo(
                                        on_wait=[w], on_update=[]
                                    ),
                                )
                            )
                        ins.sync_info = mybir.SyncInfo(
                            on_wait=keep, on_update=list(si.on_update)
                        )
                new.append(ins)
            blk.instructions[:] = new
    return n


def classify_mask(maskT):
    """Block-classify the transposed mask at 128x128 granularity.
    Returns (cls[i,j] in {0 empty,1 full,2 partial}, bias index map,
    list of multiplicative bf16 bias blocks, deduped)."""
    nb = S // 128
    cls = np.empty((nb, nb), dtype=np.int8)
    bidx = np.full((nb, nb), -1, dtype=np.int32)
    biases = []
    seen = {}
    for i in range(nb):
        for j in range(nb):
            blk = maskT[i * 128 : (i + 1) * 128, j * 128 : (j + 1) * 128]
            if (blk != 0).all():
                cls[i, j] = 1
            elif (blk == 0).all():
                cls[i, j] = 0
            else:
                cls[i, j] = 2
                m = (blk != 0).astype(np.float32)
                key = m.tobytes()
                if key not in seen:
                    seen[key] = len(biases)
                    biases.append(m)
                bidx[i, j] = seen[key]
    return cls, bidx, biases


def build_program(cls, bidx, n_bias):
    nb_alloc = max(1, n_bias)
    nc = bass.Bass("TRN2", target_bir_lowering=False, debug=False,
                   num_devices=NCORES)
    xq_d = nc.dram_tensor("xqP", [128, KT * S], bf16, kind="ExternalInput").ap()
    xk_d = nc.dram_tensor("xkP", [128, KT * S], bf16, kind="ExternalInput").ap()
    xv_d = nc.dram_tensor("xvP", [128, KT * S], bf16, kind="ExternalInput").ap()
    wq_d = nc.dram_tensor("wqP", [128, KT * DHG], bf16,
                          kind="ExternalInput").ap()
    wk_d = nc.dram_tensor("wkP", [128, KT * DHG], bf16,
                          kind="ExternalInput").ap()
    wv_d = nc.dram_tensor("wvP", [128, KT * DHG], bf16,
                          kind="ExternalInput").ap()
    wo_d = nc.dram_tensor("woP", [128, 2 * D], bf16, kind="ExternalInput").ap()
    bias_d = nc.dram_tensor("biasP", [128, nb_alloc * 128], bf16,
                            kind="ExternalInput").ap()
    y_d = nc.dram_tensor("yP", [128, ST128 * D], bf16,
                         kind="ExternalOutput").ap()

    # Every matmul is K=128, M=128, bf16 -- the PE pays a ~400ns pipeline
    # reconfig whenever consecutive matmuls change K/M/dtype, so scores use
    # per-head K-padded keys (zero rows kill the other head sharing the
    # partition range) and V is padded to 128 columns with ONES in columns
    # 64:128, which replicates the softmax denominator onto partitions
    # 64:128 of the PV accumulator for free.
    #
    # Phase A computes only K and V (chunk-major over the contraction, so
    # matmuls start when x-chunk 0 lands) plus Q for strip 0; the Q
    # projection for strip qs+1 runs INSIDE phase B's strip qs as PE filler
    # for the exp-paced attention pipeline.
    with tile.TileContext(nc) as tc:
        with tc.tile_pool(name="persist", bufs=1) as pp, tc.tile_pool(
            name="wp", bufs=1
        ) as wp:
            qt_sb = pp.tile([128, 2, S], bf16)             # Q^T head pairs
            ktp_sb = pp.tile([128, HG, S], bf16)           # K^T padded/head
            v_sb = pp.tile([128, ST128, HG, 128], bf16)    # V | ones
            ot_sb = pp.tile([128, 2, S], bf16)             # attn out^T
            wo_sb = pp.tile([128, 2, D], bf16)
            bias_sb = pp.tile([128, nb_alloc, 128], bf16)
            xq_sb = pp.tile([128, KT, S], bf16)            # persists into B

            # weight/bias DMAs from the ACT queue so the Sync queue starts
            # streaming xk chunks immediately
            wts = {}
            for which, w_d in (("k", wk_d), ("q", wq_d), ("v", wv_d)):
                wt = wp.tile([128, KT, DHG], bf16, tag=f"w{which}")
                wts[which] = wt
                nc.scalar.dma_start(
                    out=wt[:].rearrange("p a b -> p (a b)"), in_=w_d[:]
                )
            nc.scalar.dma_start(
                out=wo_sb[:].rearrange("p a b -> p (a b)"), in_=wo_d[:]
            )
            if n_bias:
                nc.scalar.dma_start(
                    out=bias_sb[:].rearrange("p a b -> p (a b)"), in_=bias_d[:]
                )
            # zero the partition ranges of ktp that K copies won't write
            # (head h lives at partitions 64*(h%2) .. +64 of slot h)
            for h in range(HG):
                po = 64 * (h % 2)
                nc.vector.memset(ktp_sb[64 - po : 128 - po, h, :], 0.0)
            # ones pad -> PV replicates the denominator over partitions 64+
            nc.vector.memset(v_sb[:, :, :, DK:128], 1.0)

            def qproj_copies(mt, qs, ps):
                q0 = qs * QS
                nc.scalar.copy(
                    out=qt_sb[:, mt, q0 : q0 + 256], in_=ps[:, :256]
                )
                nc.vector.tensor_copy(
                    out=qt_sb[:, mt, q0 + 256 : q0 + QS], in_=ps[:, 256:]
                )

            # ---- Phase A: K and V projections (+ Q strip 0) ----
            with tc.tile_pool(name="xp", bufs=2) as xp, tc.tile_pool(
                name="psA", bufs=8, space="PSUM"
            ) as psA:
                for which, x_d in (("k", xk_d), ("v", xv_d)):
                    wt = wts[which]
                    xt = xp.tile([128, KT, S], bf16, tag="xT",
                                 name=f"xt{which}")
                    for kt in range(KT):
                        nc.sync.dma_start(
                            out=xt[:, kt, :], in_=x_d[:, kt * S : (kt + 1) * S]
                        )
                    if which == "k":
                        accs = [
                            psA.tile([128, QS], f32, tag="pa",
                                     name=f"pa{which}{i}")
                            for i in range(2 * NQS)
                        ]
                        for kt in range(KT):
                            for mt in range(2):
                                for qs in range(NQS):
                                    nc.tensor.matmul(
                                        accs[2 * qs + mt][:],
                                        wt[:, kt, mt * 128 : (mt + 1) * 128],
                                        xt[:, kt, qs * QS : (qs + 1) * QS],
                                        start=(kt == 0),
                                        stop=(kt == KT - 1),
                                    )
                                    # drain each accumulator right after its
                                    # stop, split ACT/DVE so the slot frees
                                    # before the next projection needs it
                                    if kt == KT - 1:
                                        ps = accs[2 * qs + mt]
                                        q0 = qs * QS
                                        for hh in range(2):
                                            h = 2 * mt + hh
                                            po = 64 * hh
                                            eng = (
                                                nc.scalar.copy
                                                if hh == 0
                                                else nc.vector.tensor_copy
                                            )
                                            eng(
                                                out=ktp_sb[
                                                    po : po + 64, h,
                                                    q0 : q0 + QS,
                                                ],
                                                in_=ps[po : po + 64, :],
                                            )
                    else:
                        for half in range(2):
                            accs = [
                                psA.tile([128, QS], f32, tag="pa",
                                         name=f"pav{half}{i}")
                                for i in range(8)
                            ]
                            for kt in range(KT):
                                for i in range(8):
                                    st = half * 8 + i
                                    nc.tensor.matmul(
                                        accs[i][:, :DHG],
                                        xt[:, kt, st * 128 : (st + 1) * 128],
                                        wt[:, kt, :],
                                        start=(kt == 0),
                                        stop=(kt == KT - 1),
                                    )
                                    if kt == KT - 1:
                                        ps = accs[i]
                                        nc.scalar.copy(
                                            out=v_sb[:, st, 0:2, 0:DK],
                                            in_=ps[:, 0:128].rearrange(
                                                "p (h d) -> p h d", h=2
                                            ),
                                        )
                                        nc.vector.tensor_copy(
                                            out=v_sb[:, st, 2:4, 0:DK],
                                            in_=ps[:, 128:256].rearrange(
                                                "p (h d) -> p h d", h=2
                                            ),
                                        )
                # xq lands last on the Sync queue: Q strip 0 runs at the
                # end of phase A, strips 1..3 inside B
                for kt in range(KT):
                    nc.sync.dma_start(
                        out=xq_sb[:, kt, :], in_=xq_d[:, kt * S : (kt + 1) * S]
                    )
                wtq = wts["q"]
                for mt in range(2):
                    acc = psA.tile([128, QS], f32, tag="pa", name=f"paq{mt}")
                    for kt in range(KT):
                        nc.tensor.matmul(
                            acc[:],
                            wtq[:, kt, mt * 128 : (mt + 1) * 128],
                            xq_sb[:, kt, 0:QS],
                            start=(kt == 0),
                            stop=(kt == KT - 1),
                        )
                    qproj_copies(mt, 0, acc)

            # ---- Phase B: attention in head-pair passes, Q-proj + y-proj
            # interleaved as PE filler ----
            with tc.tile_pool(name="pb", bufs=4) as pb, tc.tile_pool(
                name="bc", bufs=4
            ) as bcp, tc.tile_pool(
                name="yp", bufs=3
            ) as yp, tc.tile_pool(
                name="psS", bufs=2, space="PSUM"
            ) as psS, tc.tile_pool(
                name="psOT", bufs=4, space="PSUM"
            ) as psOT:

                def emit_norm(h, pc, hqs):
                    po = 64 * (h % 2)
                    mt = h // 2
                    # 1/d = exp(-ln d): Ln and Exp share an ACT table, so
                    # no table swaps; interleaved into the exp stream
                    lt = bcp.tile([DK, QS], f32, tag="lt", name=f"lt{h}")
                    nc.scalar.activation(lt[:], pc[DK : 2 * DK, :], LN)
                    rec = bcp.tile([DK, QS], bf16, tag="rec", name=f"rec{h}")
                    nc.scalar.activation(rec[:], lt[:], EXP, scale=-1.0)
                    nc.vector.tensor_mul(
                        ot_sb[po : po + 64, mt, hqs * QS : (hqs + 1) * QS],
                        pc[0:DK, :],
                        rec[:],
                    )

                def emit_yproj(st):
                    ps = psS.tile([128, 2 * QS], f32, tag="ps", name=f"py{st}")
                    for nh in range(2):
                        for mt in range(2):
                            nc.tensor.matmul(
                                ps[:, nh * QS : (nh + 1) * QS],
                                ot_sb[:, mt, st * 128 : (st + 1) * 128],
                                wo_sb[:, mt, nh * QS : (nh + 1) * QS],
                                start=(mt == 0),
                                stop=(mt == 1),
                            )
                    y_sb = yp.tile([128, D], bf16, tag="y", name=f"ysb{st}")
                    nc.scalar.copy(out=y_sb[:, : D // 2], in_=ps[:, : D // 2])
                    nc.vector.tensor_copy(
                        out=y_sb[:, D // 2 :], in_=ps[:, D // 2 :]
                    )
                    nc.sync.dma_start(
                        out=y_d[:, st * D : (st + 1) * D], in_=y_sb[:]
                    )

                pend_norm = []
                pend_y = []
                for qs in range(NQS):
                    sub_all = cls[:, 4 * qs : 4 * qs + 4]
                    kts = [i for i in range(ST128) if sub_all[i].any()]
                    if qs:
                        pend_y.extend(
                            (qs - 1) * (QS // 128) + i
                            for i in range(QS // 128)
                        )
                    for mt in range(2):
                        # Q projection for strip qs+1 spread through pass 1
                        # as PE filler (two 1-bank accumulators in the psOT
                        # rotation)
                        qp = None
                        if mt == 1 and qs + 1 < NQS:
                            qp = [
                                psOT.tile([128, QS], f32, tag="pot",
                                          name=f"qp{qs}{m}")
                                for m in range(2)
                            ]
                            qp_kt = 0
                        pots = {
                            hh: psOT.tile([128, QS], f32, tag="pot",
                                          name=f"pot{qs}{mt}{hh}")
                            for hh in range(2)
                        }
                        def emit_pv(unit):
                            idx, kt, c0, p_sb = unit
                            for hh in range(2):
                                if idx == 0 and c0 > 0:
                                    nc.vector.memset(pots[hh][:, 0:c0], 0.0)
                                nc.tensor.matmul(
                                    pots[hh][:, c0:],
                                    v_sb[:, kt, 2 * mt + hh, :],
                                    p_sb[:, hh * QS + c0 : (hh + 1) * QS],
                                    start=(idx == 0),
                                    stop=(idx == len(kts) - 1),
                                )

                        def emit_filler(idx):
                            nonlocal qp_kt
                            if qp is not None and qp_kt < KT:
                                for m in range(2):
                                    nc.tensor.matmul(
                                        qp[m][:],
                                        wts["q"][
                                            :, qp_kt, m * 128 : (m + 1) * 128
                                        ],
                                        xq_sb[
                                            :, qp_kt,
                                            (qs + 1) * QS : (qs + 2) * QS,
                                        ],
                                        start=(qp_kt == 0),
                                        stop=(qp_kt == KT - 1),
                                    )
                                qp_kt += 1
                            elif idx >= 1 and pend_norm:
                                emit_norm(*pend_norm.pop(0))
                            elif idx >= 1 and pend_y:
                                emit_yproj(pend_y.pop(0))

                        # scores+exp run one unit ahead of PV+filler so the
                        # ACT exp stream overlaps the PE's PV/filler work
                        # instead of alternating with it
                        inflight = []
                        for idx, kt in enumerate(kts):
                            sub = sub_all[kt]
                            nz = np.nonzero(sub)[0]
                            c0 = int(nz.min()) * 128
                            c1 = (int(nz.max()) + 1) * 128
                            partial_js = [j for j in range(4) if sub[j] == 2]
                            interior = [
                                j for j in range(4)
                                if sub[j] == 0 and c0 // 128 < j < c1 // 128
                            ]
                            ps = psS.tile([128, 2 * QS], f32, tag="ps",
                                          name=f"pp{kt}")
                            for hh in range(2):
                                h = 2 * mt + hh
                                nc.tensor.matmul(
                                    ps[:, hh * QS + c0 : hh * QS + c1],
                                    ktp_sb[:, h, kt * 128 : (kt + 1) * 128],
                                    qt_sb[
                                        :, mt, qs * QS + c0 : qs * QS + c1
                                    ],
                                    start=True,
                                    stop=True,
                                )
                            p_sb = pb.tile([128, 2 * QS], bf16, tag="p",
                                           name=f"p{kt}")
                            for j in interior:
                                for hh in range(2):
                                    nc.vector.memset(
                                        p_sb[
                                            :,
                                            hh * QS + j * 128
                                            : hh * QS + (j + 1) * 128,
                                        ],
                                        0.0,
                                    )
                            if c1 < QS:
                                for hh in range(2):
                                    nc.vector.memset(
                                        p_sb[:, hh * QS + c1 : (hh + 1) * QS],
                                        0.0,
                                    )
                            nc.scalar.activation(
                                p_sb[:].rearrange(
                                    "p (a b) -> p a b", a=2
                                )[:, :, c0:c1],
                                ps[:].rearrange(
                                    "p (a b) -> p a b", a=2
                                )[:, :, c0:c1],
                                EXP,
                                scale=0.125,
                            )
                            for j in partial_js:
                                bi = int(bidx[kt, 4 * qs + j])
                                for hh in range(2):
                                    nc.vector.tensor_mul(
                                        p_sb[
                                            :,
                                            hh * QS + j * 128
                                            : hh * QS + (j + 1) * 128,
                                        ],
                                        p_sb[
                                            :,
                                            hh * QS + j * 128
                                            : hh * QS + (j + 1) * 128,
                                        ],
                                        bias_sb[:, bi, :],
                                    )
                            inflight.append((idx, kt, c0, p_sb))
                            if len(inflight) > 1:
                                emit_pv(inflight.pop(0))
                                emit_filler(idx)
                        while inflight:
                            emit_pv(inflight.pop(0))
                            emit_filler(len(kts))
                        if qp is not None:
                            while qp_kt < KT:
                                for m in range(2):
                                    nc.tensor.matmul(
                                        qp[m][:],
                                        wts["q"][
                                            :, qp_kt, m * 128 : (m + 1) * 128
                                        ],
                                        xq_sb[
                                            :, qp_kt,
                                            (qs + 1) * QS : (qs + 2) * QS,
                                        ],
                                        start=(qp_kt == 0),
                                        stop=(qp_kt == KT - 1),
                                    )
                                qp_kt += 1
                            for m in range(2):
                                qproj_copies(m, qs + 1, qp[m])
                        # free the PV accumulators fast: copy to SBUF
                        # (partitions 64:128 hold the replicated denominator)
                        for hh in range(2):
                            h = 2 * mt + hh
                            pc = bcp.tile([128, QS], f32, tag="pc",
                                          name=f"pc{h}")
                            nc.vector.tensor_copy(
                                out=pc[:], in_=pots[hh][:]
                            )
                            pend_norm.append((h, pc, qs))
                    if qs == NQS - 1:
                        for hn in list(pend_norm):
                            pend_norm.remove(hn)
                            emit_norm(*hn)
                        for st in pend_y:
                            emit_yproj(st)
                        for sti in range(QS // 128):
                            emit_yproj(qs * (QS // 128) + sti)

    _split_waits(nc)
    return nc


_program_cache = {}


def get_program(cls, bidx, n_bias):
    key = (cls.tobytes(), bidx.tobytes(), n_bias)
    if key not in _program_cache:
        _program_cache[key] = build_program(cls, bidx, n_bias)
    return _program_cache[key]


def _perm_x(xT):
    """[D, S] -> [128, KT*S] with row p holding chunks kt*128+p."""
    return np.ascontiguousarray(
        xT.reshape(KT, 128, S).transpose(1, 0, 2).reshape(128, KT * S)
    ).astype(ml_dtypes.bfloat16)


def _perm_w(wT):
    """[D, DHG] -> [128, KT*DHG]."""
    return np.ascontiguousarray(
        wT.reshape(KT, 128, DHG).transpose(1, 0, 2).reshape(128, KT * DHG)
    ).astype(ml_dtypes.bfloat16)


def make_in_maps(q, k, v, mask, w_q, w_k, w_v, w_o, biases):
    if biases:
        bia = np.stack(biases)  # [nb, 128, 128]
    else:
        bia = np.zeros((1, 128, 128), np.float32)
    bias_arr = np.ascontiguousarray(
        bia.transpose(1, 0, 2).reshape(128, -1)
    ).astype(ml_dtypes.bfloat16)
    in_maps = []
    for c in range(NCORES):
        b, g = divmod(c, 4)
        rows = slice(g * DHG, (g + 1) * DHG)
        woT = w_o[:, rows].T  # [DHG, D]
        woP = np.ascontiguousarray(
            woT.reshape(2, 128, D).transpose(1, 0, 2).reshape(128, 2 * D)
        ).astype(ml_dtypes.bfloat16)
        in_maps.append(
            {
                "xqP": _perm_x(q[b].T),
                "xkP": _perm_x(k[b].T),
                "xvP": _perm_x(v[b].T),
                "wqP": _perm_w(w_q[rows].T),
                "wkP": _perm_w(w_k[rows].T),
                "wvP": _perm_w(w_v[rows].T),
                "woP": woP,
                "biasP": bias_arr,
            }
        )
    return in_maps


def combine_results(results):
    out = np.empty((B, S, D), np.float32)
    for b in range(B):
        acc = results[4 * b]["yP"].astype(np.float32)
        for g in range(1, 4):
            acc = acc + results[4 * b + g]["yP"].astype(np.float32)
        out[b] = acc.reshape(128, ST128, D).transpose(1, 0, 2).reshape(S, D)
    return out


def kernel(q, k, v, mask, w_q, w_k, w_v, w_o):
    q = np.asarray(q, np.float32)
    k = np.asarray(k, np.float32)
    v = np.asarray(v, np.float32)
    w_q = np.asarray(w_q, np.float32)
    w_k = np.asarray(w_k, np.float32)
    w_v = np.asarray(w_v, np.float32)
    w_o = np.asarray(w_o, np.float32)
    maskT = np.ascontiguousarray(
        np.broadcast_to(np.asarray(mask), (1, 1, S, S))[0, 0].T
    )
    cls, bidx, biases = classify_mask(maskT)
    nc = get_program(cls, bidx, len(biases))
    in_maps = make_in_maps(q, k, v, mask, w_q, w_k, w_v, w_o, biases)
    res = run_bass_kernel_spmd(nc, in_maps, list(range(NCORES)))
    return combine_results(res.results)


# revision 17
# speedup vs baseline: 1.2302x; 1.2302x over previous
"""Multi-head attention (B=2, S=2048, D=1024, H=16) on 8 trn2 NeuronCores.

Sharding: batch (2) x head-groups (4 heads each, 4 groups) = 8 cores.
Each core computes Q/K/V projections for its 4 heads on its batch,
causal-masked softmax attention, and a partial output projection
(row-sharded w_o); the host sums the 4 partials per batch.

Layout strategy: the host stages every input pre-permuted so each DMA is
128 contiguous per-partition descriptors (xP[p, kt, s] = x.T[kt*128+p, s]),
and x is transferred in 8 contraction chunks so the chunk-major projection
matmuls start as soon as chunk 0 lands.  Attention scores are computed
transposed (ST[k, q]) so P = exp(ST) feeds the PV matmul directly, and V's
64 pad columns are ALL ones, which makes the PV matmul replicate the
softmax denominator across partitions 64:128 -- normalization is then just
reciprocal_approx_fast + tensor_mul, no cross-partition broadcast.
PSUM->SBUF copies are split between the Scalar and Vector engines so the
Activation engine's only phase-B job is exp (the pipeline pacer); y is
written back in bf16 per 128-row tile.
"""
import sys

sys.path.insert(0, "/opt/trn_rl_repo")

import numpy as np
import ml_dtypes

import concourse.bass as bass
import concourse.mybir as mybir
import concourse.tile as tile
from concourse.bass_utils import run_bass_kernel_spmd

B, S, D, H, DK = 2, 2048, 1024, 16, 64
NCORES = 8
HG = 4                # heads per core
DHG = HG * DK         # 256 head-dims per core
KT = D // 128         # 8 contraction chunks for the projections
ST128 = S // 128      # 16 128-row tiles of S
QS = 512              # q-strip width
NQS = S // QS         # 4 strips

f32 = mybir.dt.float32
bf16 = mybir.dt.bfloat16
EXP = mybir.ActivationFunctionType.Exp
LN = mybir.ActivationFunctionType.Ln


def _split_waits(nc, max_waits=1):
    """This walrus build rejects >1 SyncWait per instruction (and >0 on
    fp32-family matmuls, which lower through the 1-wait S3_LW struct).
    Hoist excess waits onto dedicated NOPs on the same engine queue."""
    n = 0
    for fn in nc.m.functions:
        for blk in fn.blocks:
            new = []
            for ins in blk.instructions:
                si = getattr(ins, "sync_info", None)
                if si is not None and si.on_wait:
                    limit = 0 if isinstance(ins, mybir.InstMatmult) else max_waits
                    if len(si.on_wait) > limit:
                        waits = list(si.on_wait)
                        hoist = waits if limit == 0 else waits[:-limit]
                        keep = [] if limit == 0 else waits[-limit:]
                        for w in hoist:
                            n += 1
                            new.append(
                                mybir.InstNoOp(
                                    name=f"I-waitfix-{n}",
                                    engine=ins.engine,
                                    bass_nofuse=True,
                                    sync_info=mybir.SyncInfo(
                                        on_wait=[w], on_update=[]
                                    ),
                                )
                            )
                        ins.sync_info = mybir.SyncInfo(
                            on_wait=keep, on_update=list(si.on_update)
                        )
                new.append(ins)
            blk.instructions[:] = new
    return n


def classify_mask(maskT):
    """Block-classify the transposed mask at 128x128 granularity.
    Returns (cls[i,j] in {0 empty,1 full,2 partial}, bias index map,
    list of multiplicative bf16 bias blocks, deduped)."""
    nb = S // 128
    cls = np.empty((nb, nb), dtype=np.int8)
    bidx = np.full((nb, nb), -1, dtype=np.int32)
    biases = []
    seen = {}
    for i in range(nb):
        for j in range(nb):
            blk = maskT[i * 128 : (i + 1) * 128, j * 128 : (j + 1) * 128]
            if (blk != 0).all():
                cls[i, j] = 1
            elif (blk == 0).all():
                cls[i, j] = 0
            else:
                cls[i, j] = 2
                m = (blk != 0).astype(np.float32)
                key = m.tobytes()
                if key not in seen:
                    seen[key] = len(biases)
                    biases.append(m)
                bidx[i, j] = seen[key]
    return cls, bidx, biases


def build_program(cls, bidx, n_bias):
    nb_alloc = max(1, n_bias)
    nc = bass.Bass("TRN2", target_bir_lowering=False, debug=False,
                   num_devices=NCORES)
    xq_d = nc.dram_tensor("xqP", [128, KT * S], bf16, kind="ExternalInput").ap()
    xk_d = nc.dram_tensor("xkP", [128, KT * S], bf16, kind="ExternalInput").ap()
    xv_d = nc.dram_tensor("xvP", [128, KT * S], bf16, kind="ExternalInput").ap()
    wq_d = nc.dram_tensor("wqP", [128, KT * DHG], bf16,
                          kind="ExternalInput").ap()
    wk_d = nc.dram_tensor("wkP", [128, KT * DHG], bf16,
                          kind="ExternalInput").ap()
    wv_d = nc.dram_tensor("wvP", [128, KT * DHG], bf16,
                          kind="ExternalInput").ap()
    wo_d = nc.dram_tensor("woP", [128, 2 * D], bf16, kind="ExternalInput").ap()
    bias_d = nc.dram_tensor("biasP", [128, nb_alloc * 128], bf16,
                            kind="ExternalInput").ap()
    y_d = nc.dram_tensor("yP", [128, ST128 * D], bf16,
                         kind="ExternalOutput").ap()

    # Every matmul is K=128, M=128, bf16 -- the PE pays a ~400ns pipeline
    # reconfig whenever consecutive matmuls change K/M/dtype, so scores use
    # per-head K-padded keys (zero rows kill the other head sharing the
    # partition range) and V is padded to 128 columns with ONES in columns
    # 64:128, which replicates the softmax denominator onto partitions
    # 64:128 of the PV accumulator for free.
    #
    # Phase A computes only K and V (chunk-major over the contraction, so
    # matmuls start when x-chunk 0 lands) plus Q for strip 0; the Q
    # projection for strip qs+1 runs INSIDE phase B's strip qs as PE filler
    # for the exp-paced attention pipeline.
    with tile.TileContext(nc) as tc:
        with tc.tile_pool(name="persist", bufs=1) as pp, tc.tile_pool(
            name="wp", bufs=1
        ) as wp:
            qt_sb = pp.tile([128, 2, S], bf16)             # Q^T head pairs
            ktp_sb = pp.tile([128, HG, S], bf16)           # K^T padded/head
            v_sb = pp.tile([128, ST128, HG, 128], bf16)    # V | ones
            ot_sb = pp.tile([128, 2, S], bf16)             # attn out^T
            wo_sb = pp.tile([128, 2, D], bf16)
            bias_sb = pp.tile([128, nb_alloc, 128], bf16)
            xq_sb = pp.tile([128, KT, S], bf16)            # persists into B

            # weight/bias DMAs from the ACT queue so the Sync queue starts
            # streaming xk chunks immediately
            wts = {}
            for which, w_d in (("k", wk_d), ("q", wq_d), ("v", wv_d)):
                wt = wp.tile([128, KT, DHG], bf16, tag=f"w{which}")
                wts[which] = wt
                nc.scalar.dma_start(
                    out=wt[:].rearrange("p a b -> p (a b)"), in_=w_d[:]
                )
            nc.scalar.dma_start(
                out=wo_sb[:].rearrange("p a b -> p (a b)"), in_=wo_d[:]
            )
            if n_bias:
                nc.scalar.dma_start(
                    out=bias_sb[:].rearrange("p a b -> p (a b)"), in_=bias_d[:]
                )
            # zero the partition ranges of ktp that K copies won't write
            # (head h lives at partitions 64*(h%2) .. +64 of slot h)
            for h in range(HG):
                po = 64 * (h % 2)
                nc.vector.memset(ktp_sb[64 - po : 128 - po, h, :], 0.0)
            # ones pad -> PV replicates the denominator over partitions 64+
            nc.vector.memset(v_sb[:, :, :, DK:128], 1.0)

            def qproj_copies(mt, qs, ps):
                q0 = qs * QS
                nc.scalar.copy(
                    out=qt_sb[:, mt, q0 : q0 + 256], in_=ps[:, :256]
                )
                nc.vector.tensor_copy(
                    out=qt_sb[:, mt, q0 + 256 : q0 + QS], in_=ps[:, 256:]
                )

            # ---- Phase A: K and V projections (+ Q strip 0) ----
            with tc.tile_pool(name="xp", bufs=2) as xp, tc.tile_pool(
                name="psA", bufs=8, space="PSUM"
            ) as psA:
                for which, x_d in (("k", xk_d), ("v", xv_d)):
                    wt = wts[which]
                    xt = xp.tile([128, KT, S], bf16, tag="xT",
                                 name=f"xt{which}")
                    for kt in range(KT):
                        nc.sync.dma_start(
                            out=xt[:, kt, :], in_=x_d[:, kt * S : (kt + 1) * S]
                        )
                    if which == "k":
                        accs = [
                            psA.tile([128, QS], f32, tag="pa",
                                     name=f"pa{which}{i}")
                            for i in range(2 * NQS)
                        ]
                        for kt in range(KT):
                            for mt in range(2):
                                for qs in range(NQS):
                                    nc.tensor.matmul(
                                        accs[2 * qs + mt][:],
                                        wt[:, kt, mt * 128 : (mt + 1) * 128],
                                        xt[:, kt, qs * QS : (qs + 1) * QS],
                                        start=(kt == 0),
                                        stop=(kt == KT - 1),
                                    )
                                    # drain each accumulator right after its
                                    # stop, split ACT/DVE so the slot frees
                                    # before the next projection needs it
                                    if kt == KT - 1:
                                        ps = accs[2 * qs + mt]
                                        q0 = qs * QS
                                        for hh in range(2):
                                            h = 2 * mt + hh
                                            po = 64 * hh
                                            eng = (
                                                nc.scalar.copy
                                                if hh == 0
                                                else nc.vector.tensor_copy
                                            )
                                            eng(
                                                out=ktp_sb[
                                                    po : po + 64, h,
                                                    q0 : q0 + QS,
                                                ],
                                                in_=ps[po : po + 64, :],
                                            )
                    else:
                        for half in range(2):
                            accs = [
                                psA.tile([128, QS], f32, tag="pa",
                                         name=f"pav{half}{i}")
                                for i in range(8)
                            ]
                            for kt in range(KT):
                                for i in range(8):
                                    st = half * 8 + i
                                    nc.tensor.matmul(
                                        accs[i][:, :DHG],
                                        xt[:, kt, st * 128 : (st + 1) * 128],
                                        wt[:, kt, :],
                                        start=(kt == 0),
                                        stop=(kt == KT - 1),
                                    )
                                    if kt == KT - 1:
                                        ps = accs[i]
                                        nc.scalar.copy(
                                            out=v_sb[:, st, 0:2, 0:DK],
                                            in_=ps[:, 0:128].rearrange(
                                                "p (h d) -> p h d", h=2
                                            ),
                                        )
                                        nc.vector.tensor_copy(
                                            out=v_sb[:, st, 2:4, 0:DK],
                                            in_=ps[:, 128:256].rearrange(
                                                "p (h d) -> p h d", h=2
                                            ),
                                        )
                # xq lands last on the Sync queue: Q strip 0 runs at the
                # end of phase A, strips 1..3 inside B
                for kt in range(KT):
                    nc.sync.dma_start(
                        out=xq_sb[:, kt, :], in_=xq_d[:, kt * S : (kt + 1) * S]
                    )
                wtq = wts["q"]
                for mt in range(2):
                    acc = psA.tile([128, QS], f32, tag="pa", name=f"paq{mt}")
                    for kt in range(KT):
                        nc.tensor.matmul(
                            acc[:],
                            wtq[:, kt, mt * 128 : (mt + 1) * 128],
                            xq_sb[:, kt, 0:QS],
                            start=(kt == 0),
                            stop=(kt == KT - 1),
                        )
                    qproj_copies(mt, 0, acc)

            # ---- Phase B: attention in head-pair passes, Q-proj + y-proj
            # interleaved as PE filler ----
            with tc.tile_pool(name="pb", bufs=4) as pb, tc.tile_pool(
                name="bc", bufs=4
            ) as bcp, tc.tile_pool(
                name="yp", bufs=3
            ) as yp, tc.tile_pool(
                name="psS", bufs=2, space="PSUM"
            ) as psS, tc.tile_pool(
                name="psOT", bufs=4, space="PSUM"
            ) as psOT:

                def emit_norm(h, pc, hqs):
                    po = 64 * (h % 2)
                    mt = h // 2
                    # 1/d = exp(-ln d): Ln and Exp share an ACT table, so
                    # no table swaps; interleaved into the exp stream
                    lt = bcp.tile([DK, QS], f32, tag="lt", name=f"lt{h}")
                    nc.scalar.activation(lt[:], pc[DK : 2 * DK, :], LN)
                    rec = bcp.tile([DK, QS], bf16, tag="rec", name=f"rec{h}")
                    nc.scalar.activation(rec[:], lt[:], EXP, scale=-1.0)
                    nc.vector.tensor_mul(
                        ot_sb[po : po + 64, mt, hqs * QS : (hqs + 1) * QS],
                        pc[0:DK, :],
                        rec[:],
                    )

                def emit_yproj(st):
                    ps = psS.tile([128, 2 * QS], f32, tag="ps", name=f"py{st}")
                    for nh in range(2):
                        for mt in range(2):
                            nc.tensor.matmul(
                                ps[:, nh * QS : (nh + 1) * QS],
                                ot_sb[:, mt, st * 128 : (st + 1) * 128],
                                wo_sb[:, mt, nh * QS : (nh + 1) * QS],
                                start=(mt == 0),
                                stop=(mt == 1),
                            )
                    y_sb = yp.tile([128, D], bf16, tag="y", name=f"ysb{st}")
                    nc.vector.tensor_copy(out=y_sb[:], in_=ps[:])
                    nc.sync.dma_start(
                        out=y_d[:, st * D : (st + 1) * D], in_=y_sb[:]
                    )

                pend_norm = []
                pend_y = []
                for qs in range(NQS):
                    sub_all = cls[:, 4 * qs : 4 * qs + 4]
                    kts = [i for i in range(ST128) if sub_all[i].any()]
                    if qs:
                        pend_y.extend(
                            (qs - 1) * (QS // 128) + i
                            for i in range(QS // 128)
                        )
                    for mt in range(2):
                        # Q projection for strip qs+1 spread through pass 1
                        # as PE filler (two 1-bank accumulators in the psOT
                        # rotation)
                        qp = None
                        if mt == 1 and qs + 1 < NQS:
                            qp = [
                                psOT.tile([128, QS], f32, tag="pot",
                                          name=f"qp{qs}{m}")
                                for m in range(2)
                            ]
                            qp_kt = 0
                        pots = {
                            hh: psOT.tile([128, QS], f32, tag="pot",
                                          name=f"pot{qs}{mt}{hh}")
                            for hh in range(2)
                        }
                        def emit_pv(unit):
                            idx, kt, c0, p_sb = unit
                            for hh in range(2):
                                if idx == 0 and c0 > 0:
                                    nc.vector.memset(pots[hh][:, 0:c0], 0.0)
                                nc.tensor.matmul(
                                    pots[hh][:, c0:],
                                    v_sb[:, kt, 2 * mt + hh, :],
                                    p_sb[:, hh * QS + c0 : (hh + 1) * QS],
                                    start=(idx == 0),
                                    stop=(idx == len(kts) - 1),
                                )

                        def emit_filler(idx):
                            nonlocal qp_kt
                            if qp is not None and qp_kt < KT:
                                for m in range(2):
                                    nc.tensor.matmul(
                                        qp[m][:],
                                        wts["q"][
                                            :, qp_kt, m * 128 : (m + 1) * 128
                                        ],
                                        xq_sb[
                                            :, qp_kt,
                                            (qs + 1) * QS : (qs + 2) * QS,
                                        ],
                                        start=(qp_kt == 0),
                                        stop=(qp_kt == KT - 1),
                                    )
                                qp_kt += 1
                            elif idx >= 1 and pend_norm:
                                emit_norm(*pend_norm.pop(0))
                            elif idx >= 1 and pend_y:
                                emit_yproj(pend_y.pop(0))

                        # scores+exp run one unit ahead of PV+filler so the
                        # ACT exp stream overlaps the PE's PV/filler work
                        # instead of alternating with it
                        inflight = []
                        for idx, kt in enumerate(kts):
                            sub = sub_all[kt]
                            nz = np.nonzero(sub)[0]
                            c0 = int(nz.min()) * 128
                            c1 = (int(nz.max()) + 1) * 128
                            partial_js = [j for j in range(4) if sub[j] == 2]
                            interior = [
                                j for j in range(4)
                                if sub[j] == 0 and c0 // 128 < j < c1 // 128
                            ]
                            ps = psS.tile([128, 2 * QS], f32, tag="ps",
                                          name=f"pp{kt}")
                            for hh in range(2):
                                h = 2 * mt + hh
                                nc.tensor.matmul(
                                    ps[:, hh * QS + c0 : hh * QS + c1],
                                    ktp_sb[:, h, kt * 128 : (kt + 1) * 128],
                                    qt_sb[
                                        :, mt, qs * QS + c0 : qs * QS + c1
                                    ],
                                    start=True,
                                    stop=True,
                                )
                            p_sb = pb.tile([128, 2 * QS], bf16, tag="p",
                                           name=f"p{kt}")
                            for j in interior:
                                for hh in range(2):
                                    nc.vector.memset(
                                        p_sb[
                                            :,
                                            hh * QS + j * 128
                                            : hh * QS + (j + 1) * 128,
                                        ],
                                        0.0,
                                    )
                            if c1 < QS:
                                for hh in range(2):
                                    nc.vector.memset(
                                        p_sb[:, hh * QS + c1 : (hh + 1) * QS],
                                        0.0,
                                    )
                            nc.scalar.activation(
                                p_sb[:].rearrange(
                                    "p (a b) -> p a b", a=2
                                )[:, :, c0:c1],
                                ps[:].rearrange(
                                    "p (a b) -> p a b", a=2
                                )[:, :, c0:c1],
                                EXP,
                                scale=0.125,
                            )
                            for j in partial_js:
                                bi = int(bidx[kt, 4 * qs + j])
                                for hh in range(2):
                                    nc.vector.tensor_mul(
                                        p_sb[
                                            :,
                                            hh * QS + j * 128
                                            : hh * QS + (j + 1) * 128,
                                        ],
                                        p_sb[
                                            :,
                                            hh * QS + j * 128
                                            : hh * QS + (j + 1) * 128,
                                        ],
                                        bias_sb[:, bi, :],
                                    )
                            inflight.append((idx, kt, c0, p_sb))
                            if len(inflight) > 1:
                                emit_pv(inflight.pop(0))
                                emit_filler(idx)
                        while inflight:
                            emit_pv(inflight.pop(0))
                            emit_filler(len(kts))
                        if qp is not None:
                            while qp_kt < KT:
                                for m in range(2):
                                    nc.tensor.matmul(
                                        qp[m][:],
                                        wts["q"][
                                            :, qp_kt, m * 128 : (m + 1) * 128
                                        ],
                                        xq_sb[
                                            :, qp_kt,
                                            (qs + 1) * QS : (qs + 2) * QS,
                                        ],
                                        start=(qp_kt == 0),
                                        stop=(qp_kt == KT - 1),
                                    )
                                qp_kt += 1
                            for m in range(2):
                                qproj_copies(m, qs + 1, qp[m])
                        # free the PV accumulators fast: copy to SBUF
                        # (partitions 64:128 hold the replicated denominator)
                        for hh in range(2):
                            h = 2 * mt + hh
                            pc = bcp.tile([128, QS], f32, tag="pc",
                                          name=f"pc{h}")
                            nc.vector.tensor_copy(
                                out=pc[:], in_=pots[hh][:]
                            )
                            pend_norm.append((h, pc, qs))
                    if qs == NQS - 1:
                        for hn in list(pend_norm):
                            pend_norm.remove(hn)
                            emit_norm(*hn)
                        for st in pend_y:
                            emit_yproj(st)
                        for sti in range(QS // 128):
                            emit_yproj(qs * (QS // 128) + sti)

    _split_waits(nc)
    return nc


_program_cache = {}


def get_program(cls, bidx, n_bias):
    key = (cls.tobytes(), bidx.tobytes(), n_bias)
    if key not in _program_cache:
        _program_cache[key] = build_program(cls, bidx, n_bias)
    return _program_cache[key]


def _perm_x(xT):
    """[D, S] -> [128, KT*S] with row p holding chunks kt*128+p."""
    return np.ascontiguousarray(
        xT.reshape(KT, 128, S).transpose(1, 0, 2).reshape(128, KT * S)
    ).astype(ml_dtypes.bfloat16)


def _perm_w(wT):
    """[D, DHG] -> [128, KT*DHG]."""
    return np.ascontiguousarray(
        wT.reshape(KT, 128, DHG).transpose(1, 0, 2).reshape(128, KT * DHG)
    ).astype(ml_dtypes.bfloat16)


def make_in_maps(q, k, v, mask, w_q, w_k, w_v, w_o, biases):
    if biases:
        bia = np.stack(biases)  # [nb, 128, 128]
    else:
        bia = np.zeros((1, 128, 128), np.float32)
    bias_arr = np.ascontiguousarray(
        bia.transpose(1, 0, 2).reshape(128, -1)
    ).astype(ml_dtypes.bfloat16)
    in_maps = []
    for c in range(NCORES):
        b, g = divmod(c, 4)
        rows = slice(g * DHG, (g + 1) * DHG)
        woT = w_o[:, rows].T  # [DHG, D]
        woP = np.ascontiguousarray(
            woT.reshape(2, 128, D).transpose(1, 0, 2).reshape(128, 2 * D)
        ).astype(ml_dtypes.bfloat16)
        in_maps.append(
            {
                "xqP": _perm_x(q[b].T),
                "xkP": _perm_x(k[b].T),
                "xvP": _perm_x(v[b].T),
                "wqP": _perm_w(w_q[rows].T),
                "wkP": _perm_w(w_k[rows].T),
                "wvP": _perm_w(w_v[rows].T),
                "woP": woP,
                "biasP": bias_arr,
            }
        )
    return in_maps


def combine_results(results):
    out = np.empty((B, S, D), np.float32)
    for b in range(B):
        acc = results[4 * b]["yP"].astype(np.float32)
        for g in range(1, 4):
            acc = acc + results[4 * b + g]["yP"].astype(np.float32)
        out[b] = acc.reshape(128, ST128, D).transpose(1, 0, 2).reshape(S, D)
    return out


def kernel(q, k, v, mask, w_q, w_k, w_v, w_o):
    q = np.asarray(q, np.float32)
    k = np.asarray(k, np.float32)
    v = np.asarray(v, np.float32)
    w_q = np.asarray(w_q, np.float32)
    w_k = np.asarray(w_k, np.float32)
    w_v = np.asarray(w_v, np.float32)
    w_o = np.asarray(w_o, np.float32)
    maskT = np.ascontiguousarray(
        np.broadcast_to(np.asarray(mask), (1, 1, S, S))[0, 0].T
    )
    cls, bidx, biases = classify_mask(maskT)
    nc = get_program(cls, bidx, len(biases))
    in_maps = make_in_maps(q, k, v, mask, w_q, w_k, w_v, w_o, biases)
    res = run_bass_kernel_spmd(nc, in_maps, list(range(NCORES)))
    return combine_results(res.results)


# revision 20
# speedup vs baseline: 1.2572x; 1.0220x over previous
"""Multi-head attention (B=2, S=2048, D=1024, H=16) on 8 trn2 NeuronCores.

Sharding: batch (2) x head-groups (4 heads each, 4 groups) = 8 cores.
Each core computes Q/K/V projections for its 4 heads on its batch,
causal-masked softmax attention, and a partial output projection
(row-sharded w_o); the host sums the 4 partials per batch.

Layout strategy: the host stages every input pre-permuted so each DMA is
128 contiguous per-partition descriptors (xP[p, kt, s] = x.T[kt*128+p, s]),
and x is transferred in 8 contraction chunks so the chunk-major projection
matmuls start as soon as chunk 0 lands.  Attention scores are computed
transposed (ST[k, q]) so P = exp(ST) feeds the PV matmul directly, and V's
64 pad columns are ALL ones, which makes the PV matmul replicate the
softmax denominator across partitions 64:128 -- normalization is then just
reciprocal_approx_fast + tensor_mul, no cross-partition broadcast.
PSUM->SBUF copies are split between the Scalar and Vector engines so the
Activation engine's only phase-B job is exp (the pipeline pacer); y is
written back in bf16 per 128-row tile.
"""
import sys

sys.path.insert(0, "/opt/trn_rl_repo")

import numpy as np
import ml_dtypes

import concourse.bass as bass
import concourse.mybir as mybir
import concourse.tile as tile
from concourse.bass_utils import run_bass_kernel_spmd

B, S, D, H, DK = 2, 2048, 1024, 16, 64
NCORES = 8
HG = 4                # heads per core
DHG = HG * DK         # 256 head-dims per core
KT = D // 128         # 8 contraction chunks for the projections
ST128 = S // 128      # 16 128-row tiles of S
QS = 512              # q-strip width
NQS = S // QS         # 4 strips

f32 = mybir.dt.float32
bf16 = mybir.dt.bfloat16
EXP = mybir.ActivationFunctionType.Exp
LN = mybir.ActivationFunctionType.Ln


def _split_waits(nc, max_waits=1):
    """This walrus build rejects >1 SyncWait per instruction (and >0 on
    fp32-family matmuls, which lower through the 1-wait S3_LW struct).
    Hoist excess waits onto dedicated NOPs on the same engine queue."""
    n = 0
    for fn in nc.m.functions:
        for blk in fn.blocks:
            new = []
            for ins in blk.instructions:
                si = getattr(ins, "sync_info", None)
                if si is not None and si.on_wait:
                    limit = 0 if isinstance(ins, mybir.InstMatmult) else max_waits
                    if len(si.on_wait) > limit:
                        waits = list(si.on_wait)
                        hoist = waits if limit == 0 else waits[:-limit]
                        keep = [] if limit == 0 else waits[-limit:]
                        for w in hoist:
                            n += 1
                            new.append(
                                mybir.InstNoOp(
                                    name=f"I-waitfix-{n}",
                                    engine=ins.engine,
                                    bass_nofuse=True,
                                    sync_info=mybir.SyncInfo(
                                        on_wait=[w], on_update=[]
                                    ),
                                )
                            )
                        ins.sync_info = mybir.SyncInfo(
                            on_wait=keep, on_update=list(si.on_update)
                        )
                new.append(ins)
            blk.instructions[:] = new
    return n


def classify_mask(maskT):
    """Block-classify the transposed mask at 128x128 granularity.
    Returns (cls[i,j] in {0 empty,1 full,2 partial}, bias index map,
    list of multiplicative bf16 bias blocks, deduped)."""
    nb = S // 128
    cls = np.empty((nb, nb), dtype=np.int8)
    bidx = np.full((nb, nb), -1, dtype=np.int32)
    biases = []
    seen = {}
    for i in range(nb):
        for j in range(nb):
            blk = maskT[i * 128 : (i + 1) * 128, j * 128 : (j + 1) * 128]
            if (blk != 0).all():
                cls[i, j] = 1
            elif (blk == 0).all():
                cls[i, j] = 0
            else:
                cls[i, j] = 2
                m = (blk != 0).astype(np.float32)
                key = m.tobytes()
                if key not in seen:
                    seen[key] = len(biases)
                    biases.append(m)
                bidx[i, j] = seen[key]
    return cls, bidx, biases


def build_program(cls, bidx, n_bias):
    nb_alloc = max(1, n_bias)
    nc = bass.Bass("TRN2", target_bir_lowering=False, debug=False,
                   num_devices=NCORES)
    xq_d = nc.dram_tensor("xqP", [128, KT * S], bf16, kind="ExternalInput").ap()
    xk_d = nc.dram_tensor("xkP", [128, KT * S], bf16, kind="ExternalInput").ap()
    xv_d = nc.dram_tensor("xvP", [128, KT * S], bf16, kind="ExternalInput").ap()
    wq_d = nc.dram_tensor("wqP", [128, KT * DHG], bf16,
                          kind="ExternalInput").ap()
    wk_d = nc.dram_tensor("wkP", [128, KT * DHG], bf16,
                          kind="ExternalInput").ap()
    wv_d = nc.dram_tensor("wvP", [128, KT * DHG], bf16,
                          kind="ExternalInput").ap()
    wo_d = nc.dram_tensor("woP", [128, 2 * D], bf16, kind="ExternalInput").ap()
    bias_d = nc.dram_tensor("biasP", [128, nb_alloc * 128], bf16,
                            kind="ExternalInput").ap()
    y_d = nc.dram_tensor("yP", [128, ST128 * D], bf16,
                         kind="ExternalOutput").ap()

    # Every matmul is K=128, M=128, bf16 -- the PE pays a ~400ns pipeline
    # reconfig whenever consecutive matmuls change K/M/dtype, so scores use
    # per-head K-padded keys (zero rows kill the other head sharing the
    # partition range) and V is padded to 128 columns with ONES in columns
    # 64:128, which replicates the softmax denominator onto partitions
    # 64:128 of the PV accumulator for free.
    #
    # Phase A computes only K and V (chunk-major over the contraction, so
    # matmuls start when x-chunk 0 lands) plus Q for strip 0; the Q
    # projection for strip qs+1 runs INSIDE phase B's strip qs as PE filler
    # for the exp-paced attention pipeline.
    with tile.TileContext(nc) as tc:
        with tc.tile_pool(name="persist", bufs=1) as pp, tc.tile_pool(
            name="wp", bufs=1
        ) as wp:
            qt_sb = pp.tile([128, 2, S], bf16)             # Q^T head pairs
            ktp_sb = pp.tile([128, HG, S], bf16)           # K^T padded/head
            v_sb = pp.tile([128, ST128, HG, 128], bf16)    # V | ones
            ot_sb = pp.tile([128, 2, S], bf16)             # attn out^T
            wo_sb = pp.tile([128, 2, D], bf16)
            bias_sb = pp.tile([128, nb_alloc, 128], bf16)
            xq_sb = pp.tile([128, KT, S], bf16)            # persists into B

            # weight/bias DMAs from the ACT queue so the Sync queue starts
            # streaming xk chunks immediately
            wts = {}
            for which, w_d in (("k", wk_d), ("q", wq_d), ("v", wv_d)):
                wt = wp.tile([128, KT, DHG], bf16, tag=f"w{which}")
                wts[which] = wt
                nc.scalar.dma_start(
                    out=wt[:].rearrange("p a b -> p (a b)"), in_=w_d[:]
                )
            nc.scalar.dma_start(
                out=wo_sb[:].rearrange("p a b -> p (a b)"), in_=wo_d[:]
            )
            if n_bias:
                nc.scalar.dma_start(
                    out=bias_sb[:].rearrange("p a b -> p (a b)"), in_=bias_d[:]
                )
            # zero the partition ranges of ktp that K copies won't write
            # (head h lives at partitions 64*(h%2) .. +64 of slot h)
            for h in range(HG):
                po = 64 * (h % 2)
                nc.vector.memset(ktp_sb[64 - po : 128 - po, h, :], 0.0)
            # ones pad -> PV replicates the denominator over partitions 64+
            nc.vector.memset(v_sb[:, :, :, DK:128], 1.0)

            def qproj_copies(mt, qs, ps):
                q0 = qs * QS
                nc.scalar.copy(
                    out=qt_sb[:, mt, q0 : q0 + 256], in_=ps[:, :256]
                )
                nc.vector.tensor_copy(
                    out=qt_sb[:, mt, q0 + 256 : q0 + QS], in_=ps[:, 256:]
                )

            # ---- Phase A: K and V projections (+ Q strip 0) ----
            with tc.tile_pool(name="xp", bufs=2) as xp, tc.tile_pool(
                name="psA", bufs=8, space="PSUM"
            ) as psA:
                for which, x_d in (("k", xk_d), ("v", xv_d)):
                    wt = wts[which]
                    xt = xp.tile([128, KT, S], bf16, tag="xT",
                                 name=f"xt{which}")
                    for kt in range(KT):
                        nc.sync.dma_start(
                            out=xt[:, kt, :], in_=x_d[:, kt * S : (kt + 1) * S]
                        )
                    if which == "k":
                        accs = [
                            psA.tile([128, QS], f32, tag="pa",
                                     name=f"pa{which}{i}")
                            for i in range(2 * NQS)
                        ]
                        for kt in range(KT):
                            for mt in range(2):
                                for qs in range(NQS):
                                    nc.tensor.matmul(
                                        accs[2 * qs + mt][:],
                                        wt[:, kt, mt * 128 : (mt + 1) * 128],
                                        xt[:, kt, qs * QS : (qs + 1) * QS],
                                        start=(kt == 0),
                                        stop=(kt == KT - 1),
                                    )
                                    # drain each accumulator right after its
                                    # stop, split ACT/DVE so the slot frees
                                    # before the next projection needs it
                                    if kt == KT - 1:
                                        ps = accs[2 * qs + mt]
                                        q0 = qs * QS
                                        for hh in range(2):
                                            h = 2 * mt + hh
                                            po = 64 * hh
                                            eng = (
                                                nc.scalar.copy
                                                if hh == 0
                                                else nc.vector.tensor_copy
                                            )
                                            eng(
                                                out=ktp_sb[
                                                    po : po + 64, h,
                                                    q0 : q0 + QS,
                                                ],
                                                in_=ps[po : po + 64, :],
                                            )
                    else:
                        for half in range(2):
                            accs = [
                                psA.tile([128, QS], f32, tag="pa",
                                         name=f"pav{half}{i}")
                                for i in range(8)
                            ]
                            for kt in range(KT):
                                for i in range(8):
                                    st = half * 8 + i
                                    nc.tensor.matmul(
                                        accs[i][:, :DHG],
                                        xt[:, kt, st * 128 : (st + 1) * 128],
                                        wt[:, kt, :],
                                        start=(kt == 0),
                                        stop=(kt == KT - 1),
                                    )
                                    if kt == KT - 1:
                                        ps = accs[i]
                                        nc.scalar.copy(
                                            out=v_sb[:, st, 0:2, 0:DK],
                                            in_=ps[:, 0:128].rearrange(
                                                "p (h d) -> p h d", h=2
                                            ),
                                        )
                                        nc.vector.tensor_copy(
                                            out=v_sb[:, st, 2:4, 0:DK],
                                            in_=ps[:, 128:256].rearrange(
                                                "p (h d) -> p h d", h=2
                                            ),
                                        )
                # xq lands last on the Sync queue: Q strip 0 runs at the
                # end of phase A, strips 1..3 inside B
                for kt in range(KT):
                    nc.sync.dma_start(
                        out=xq_sb[:, kt, :], in_=xq_d[:, kt * S : (kt + 1) * S]
                    )
                wtq = wts["q"]
                for mt in range(2):
                    acc = psA.tile([128, QS], f32, tag="pa", name=f"paq{mt}")
                    for kt in range(KT):
                        nc.tensor.matmul(
                            acc[:],
                            wtq[:, kt, mt * 128 : (mt + 1) * 128],
                            xq_sb[:, kt, 0:QS],
                            start=(kt == 0),
                            stop=(kt == KT - 1),
                        )
                    qproj_copies(mt, 0, acc)

            # ---- Phase B: attention in head-pair passes, Q-proj + y-proj
            # interleaved as PE filler ----
            with tc.tile_pool(name="pb", bufs=4) as pb, tc.tile_pool(
                name="bc", bufs=4
            ) as bcp, tc.tile_pool(
                name="yp", bufs=3
            ) as yp, tc.tile_pool(
                name="psS", bufs=2, space="PSUM"
            ) as psS, tc.tile_pool(
                name="psOT", bufs=4, space="PSUM"
            ) as psOT:

                def emit_norm(h, pc, hqs):
                    po = 64 * (h % 2)
                    mt = h // 2
                    # 1/d = exp(-ln d): Ln and Exp share an ACT table, so
                    # no table swaps; interleaved into the exp stream
                    lt = bcp.tile([DK, QS], f32, tag="lt", name=f"lt{h}")
                    nc.scalar.activation(lt[:], pc[DK : 2 * DK, :], LN)
                    rec = bcp.tile([DK, QS], bf16, tag="rec", name=f"rec{h}")
                    nc.scalar.activation(rec[:], lt[:], EXP, scale=-1.0)
                    nc.vector.tensor_mul(
                        ot_sb[po : po + 64, mt, hqs * QS : (hqs + 1) * QS],
                        pc[0:DK, :],
                        rec[:],
                    )

                def emit_yproj(st):
                    ps = psS.tile([128, 2 * QS], f32, tag="ps", name=f"py{st}")
                    for nh in range(2):
                        for mt in range(2):
                            nc.tensor.matmul(
                                ps[:, nh * QS : (nh + 1) * QS],
                                ot_sb[:, mt, st * 128 : (st + 1) * 128],
                                wo_sb[:, mt, nh * QS : (nh + 1) * QS],
                                start=(mt == 0),
                                stop=(mt == 1),
                            )
                    y_sb = yp.tile([128, D], bf16, tag="y", name=f"ysb{st}")
                    nc.vector.tensor_copy(out=y_sb[:], in_=ps[:])
                    nc.sync.dma_start(
                        out=y_d[:, st * D : (st + 1) * D], in_=y_sb[:]
                    )

                pend_norm = []
                pend_y = []

                # Flat software-pipelined unit stream over (strip, pass, kt):
                # scores+exp for unit u+1 are emitted before PV+filler of
                # unit u, across pass and strip boundaries, so the ACT exp
                # stream never drains at a boundary.
                units = []
                for qs in range(NQS):
                    sub_all = cls[:, 4 * qs : 4 * qs + 4]
                    kts = [i for i in range(ST128) if sub_all[i].any()]
                    for mt in range(2):
                        for idx, kt in enumerate(kts):
                            sub = sub_all[kt]
                            nz = np.nonzero(sub)[0]
                            units.append(
                                dict(
                                    qs=qs, mt=mt, idx=idx, kt=kt,
                                    c0=int(nz.min()) * 128,
                                    c1=(int(nz.max()) + 1) * 128,
                                    partial_js=[
                                        j for j in range(4) if sub[j] == 2
                                    ],
                                    interior=[
                                        j for j in range(4)
                                        if sub[j] == 0
                                        and int(nz.min()) < j < int(nz.max())
                                    ],
                                    nkts=len(kts),
                                )
                            )

                pots = {}          # (qs, mt) -> [tile, tile]
                qp_state = {}      # qs (being projected) -> [qp0, qp1, kt]

                def emit_scores_exp(u):
                    qs, mt, kt = u["qs"], u["mt"], u["kt"]
                    c0, c1 = u["c0"], u["c1"]
                    ps = psS.tile([128, 2 * QS], f32, tag="ps",
                                  name=f"pp{qs}{mt}{kt}")
                    for hh in range(2):
                        h = 2 * mt + hh
                        nc.tensor.matmul(
                            ps[:, hh * QS + c0 : hh * QS + c1],
                            ktp_sb[:, h, kt * 128 : (kt + 1) * 128],
                            qt_sb[:, mt, qs * QS + c0 : qs * QS + c1],
                            start=True,
                            stop=True,
                        )
                    p_sb = pb.tile([128, 2 * QS], bf16, tag="p",
                                   name=f"p{qs}{mt}{kt}")
                    for j in u["interior"]:
                        for hh in range(2):
                            nc.vector.memset(
                                p_sb[
                                    :,
                                    hh * QS + j * 128
                                    : hh * QS + (j + 1) * 128,
                                ],
                                0.0,
                            )
                    if c1 < QS:
                        for hh in range(2):
                            nc.vector.memset(
                                p_sb[:, hh * QS + c1 : (hh + 1) * QS], 0.0
                            )
                    nc.scalar.activation(
                        p_sb[:].rearrange("p (a b) -> p a b", a=2)[:, :, c0:c1],
                        ps[:].rearrange("p (a b) -> p a b", a=2)[:, :, c0:c1],
                        EXP,
                        scale=0.125,
                    )
                    for j in u["partial_js"]:
                        bi = int(bidx[u["kt"], 4 * qs + j])
                        for hh in range(2):
                            nc.vector.tensor_mul(
                                p_sb[
                                    :,
                                    hh * QS + j * 128
                                    : hh * QS + (j + 1) * 128,
                                ],
                                p_sb[
                                    :,
                                    hh * QS + j * 128
                                    : hh * QS + (j + 1) * 128,
                                ],
                                bias_sb[:, bi, :],
                            )
                    u["p_sb"] = p_sb

                def emit_pv(u):
                    qs, mt, idx = u["qs"], u["mt"], u["idx"]
                    if idx == 0:
                        pots[(qs, mt)] = [
                            psOT.tile([128, QS], f32, tag="pot",
                                      name=f"pot{qs}{mt}{hh}")
                            for hh in range(2)
                        ]
                    pt = pots[(qs, mt)]
                    for hh in range(2):
                        if idx == 0 and u["c0"] > 0:
                            nc.vector.memset(pt[hh][:, 0 : u["c0"]], 0.0)
                        nc.tensor.matmul(
                            pt[hh][:, u["c0"] :],
                            v_sb[:, u["kt"], 2 * mt + hh, :],
                            u["p_sb"][:, hh * QS + u["c0"] : (hh + 1) * QS],
                            start=(idx == 0),
                            stop=(idx == u["nkts"] - 1),
                        )
                    if idx == u["nkts"] - 1:
                        # pass done: free the PV accumulators via SBUF
                        # copies; the last pass normalizes from PSUM
                        # directly (nothing waits on those banks)
                        last = qs == NQS - 1 and mt == 1
                        for hh in range(2):
                            h = 2 * mt + hh
                            if last:
                                pend_norm.append((h, pt[hh], qs))
                            else:
                                pc = bcp.tile([128, QS], f32, tag="pc",
                                              name=f"pc{qs}{h}")
                                nc.vector.tensor_copy(
                                    out=pc[:], in_=pt[hh][:]
                                )
                                pend_norm.append((h, pc, qs))

                def emit_filler(u):
                    # PE filler after each PV: Q-proj chunks for the next
                    # strip (started in pass 1, drained from any later
                    # unit), else pending norms/y-projs
                    qs, mt = u["qs"], u["mt"]
                    if mt == 1 and qs + 1 < NQS and qs + 1 not in qp_state:
                        # psOT slots for the Q accumulators free up when
                        # pass 0's pc copies complete
                        qp_state[qs + 1] = [
                            psOT.tile([128, QS], f32, tag="pot",
                                      name=f"qp{qs + 1}{m}")
                            for m in range(2)
                        ] + [0]
                    for tqs, st8 in qp_state.items():
                        if st8[2] < KT:
                            qkt = st8[2]
                            for m in range(2):
                                nc.tensor.matmul(
                                    st8[m][:],
                                    wts["q"][:, qkt, m * 128 : (m + 1) * 128],
                                    xq_sb[:, qkt, tqs * QS : (tqs + 1) * QS],
                                    start=(qkt == 0),
                                    stop=(qkt == KT - 1),
                                )
                            st8[2] += 1
                            if st8[2] == KT:
                                for m in range(2):
                                    qproj_copies(m, tqs, st8[m])
                            return
                    if pend_norm:
                        emit_norm(*pend_norm.pop(0))
                    elif pend_y:
                        emit_yproj(pend_y.pop(0))

                def drain_qp(tqs):
                    st8 = qp_state.get(tqs)
                    if st8 is None or st8[2] >= KT:
                        return
                    while st8[2] < KT:
                        qkt = st8[2]
                        for m in range(2):
                            nc.tensor.matmul(
                                st8[m][:],
                                wts["q"][:, qkt, m * 128 : (m + 1) * 128],
                                xq_sb[:, qkt, tqs * QS : (tqs + 1) * QS],
                                start=(qkt == 0),
                                stop=(qkt == KT - 1),
                            )
                        st8[2] += 1
                    for m in range(2):
                        qproj_copies(m, tqs, st8[m])

                inflight = []
                for u in units:
                    if u["qs"] and u["mt"] == 0 and u["idx"] == 0:
                        # qt for this strip must be fully written before its
                        # scores are emitted (Tile deps follow program order)
                        drain_qp(u["qs"])
                        pend_y.extend(
                            (u["qs"] - 1) * (QS // 128) + i
                            for i in range(QS // 128)
                        )
                    emit_scores_exp(u)
                    inflight.append(u)
                    if len(inflight) > 1:
                        v = inflight.pop(0)
                        emit_pv(v)
                        emit_filler(v)
                while inflight:
                    v = inflight.pop(0)
                    emit_pv(v)
                    emit_filler(v)
                # tail: leftover Q-proj chunks (none expected), norms, ys
                for hn in list(pend_norm):
                    pend_norm.remove(hn)
                    emit_norm(*hn)
                for st in pend_y:
                    emit_yproj(st)
                for sti in range(QS // 128):
                    emit_yproj((NQS - 1) * (QS // 128) + sti)

    _split_waits(nc)
    return nc


_program_cache = {}


def get_program(cls, bidx, n_bias):
    key = (cls.tobytes(), bidx.tobytes(), n_bias)
    if key not in _program_cache:
        _program_cache[key] = build_program(cls, bidx, n_bias)
    return _program_cache[key]


def _perm_x(xT):
    """[D, S] -> [128, KT*S] with row p holding chunks kt*128+p."""
    return np.ascontiguousarray(
        xT.reshape(KT, 128, S).transpose(1, 0, 2).reshape(128, KT * S)
    ).astype(ml_dtypes.bfloat16)


def _perm_w(wT):
    """[D, DHG] -> [128, KT*DHG]."""
    return np.ascontiguousarray(
        wT.reshape(KT, 128, DHG).transpose(1, 0, 2).reshape(128, KT * DHG)
    ).astype(ml_dtypes.bfloat16)


def make_in_maps(q, k, v, mask, w_q, w_k, w_v, w_o, biases):
    if biases:
        bia = np.stack(biases)  # [nb, 128, 128]
    else:
        bia = np.zeros((1, 128, 128), np.float32)
    bias_arr = np.ascontiguousarray(
        bia.transpose(1, 0, 2).reshape(128, -1)
    ).astype(ml_dtypes.bfloat16)
    in_maps = []
    for c in range(NCORES):
        b, g = divmod(c, 4)
        rows = slice(g * DHG, (g + 1) * DHG)
        woT = w_o[:, rows].T  # [DHG, D]
        woP = np.ascontiguousarray(
            woT.reshape(2, 128, D).transpose(1, 0, 2).reshape(128, 2 * D)
        ).astype(ml_dtypes.bfloat16)
        in_maps.append(
            {
                "xqP": _perm_x(q[b].T),
                "xkP": _perm_x(k[b].T),
                "xvP": _perm_x(v[b].T),
                "wqP": _perm_w(w_q[rows].T),
                "wkP": _perm_w(w_k[rows].T),
                "wvP": _perm_w(w_v[rows].T),
                "woP": woP,
                "biasP": bias_arr,
            }
        )
    return in_maps


def combine_results(results):
    out = np.empty((B, S, D), np.float32)
    for b in range(B):
        acc = results[4 * b]["yP"].astype(np.float32)
        for g in range(1, 4):
            acc = acc + results[4 * b + g]["yP"].astype(np.float32)
        out[b] = acc.reshape(128, ST128, D).transpose(1, 0, 2).reshape(S, D)
    return out


def kernel(q, k, v, mask, w_q, w_k, w_v, w_o):
    q = np.asarray(q, np.float32)
    k = np.asarray(k, np.float32)
    v = np.asarray(v, np.float32)
    w_q = np.asarray(w_q, np.float32)
    w_k = np.asarray(w_k, np.float32)
    w_v = np.asarray(w_v, np.float32)
    w_o = np.asarray(w_o, np.float32)
    maskT = np.ascontiguousarray(
        np.broadcast_to(np.asarray(mask), (1, 1, S, S))[0, 0].T
    )
    cls, bidx, biases = classify_mask(maskT)
    nc = get_program(cls, bidx, len(biases))
    in_maps = make_in_maps(q, k, v, mask, w_q, w_k, w_v, w_o, biases)
    res = run_bass_kernel_spmd(nc, in_maps, list(range(NCORES)))
    return combine_results(res.results)
